# revision 12
# baseline (speedup 1.0000x reference)
"""GroupQuantLinear: y = x @ dequant(w).T + b on 8 NeuronCores.

Strategy (column-parallel, fp8 DoubleRow PE path):
  The 4-bit weight W = n*s + t (nibble n in 0..15, per-group scale s, bias t)
  is split as W = (n-7.5)*s + (7.5*s + t).
    - Residual Wq = (n-7.5)*s carries only ~14% of W's variance. It is cast
      to fp8 e4m3 with a per-output-column power-of-2 scale A[o] (exactly
      invertible) and multiplied against fp8(x) using DoubleRow matmuls:
      2 K-slices (K=256) per PE instruction at 1 col/cycle -> 2x fp16 rate.
      Combined fp8 quantization error ~3.8% * sqrt(0.14) = ~1.4% rel.
    - The group-affine part y_aug = xg @ (7.5s+t).T + b (xg = per-group sums
      of x) is 1.5% of the FLOPs and exact; computed on host in fp32,
      shipped pre-scaled by A as fp16, and added to PSUM by the DVE during
      eject. Device returns A*y in fp32; host multiplies by 1/A.
  All DRAM tensors are laid out host-side so DMA descriptors are multi-KB
  contiguous runs (the DMA engines are descriptor-rate-bound ~40ns each).
  PE cost per (token-tile, chunk): 16 DoubleRow matmuls = 16N cycles vs
  32N for fp16 - the pure-fp8 PE floor (~587us/core).
  Shards W/outputs along out_features across 8 cores (1376 each).
"""

import os
import sys
from contextlib import ExitStack

import numpy as np

sys.path.insert(0, "/opt/trn_rl_repo")

TOKENS = 8192
IN_F = 4096
OUT_F = 11008
N_CORES = 8
SHARD = OUT_F // N_CORES          # 1376
CHUNKS = (512, 512, 352)          # out-cols per PSUM bank, sum = SHARD
P = 128
KS = IN_F // P                    # 32 k-slices
DR = KS // 2                      # 16 DoubleRow steps
TT = TOKENS // P                  # 64 token tiles
NG = 64                           # quant groups

_NC_CACHE = {}


def _build_nc():
    import concourse.bacc as bacc
    import concourse.mybir as mybir
    import concourse.tile as tile

    dt8 = mybir.dt.float8e4
    dt16 = mybir.dt.float16
    f32 = mybir.dt.float32
    DRMODE = mybir.MatmulPerfMode.DoubleRow

    nc = bacc.Bacc(
        "TRN2",
        target_bir_lowering=False,
        debug=False,
        enable_asserts=False,
        num_devices=N_CORES,
    )
    # tile-blocked layouts: contiguous multi-KB runs per partition
    xt = nc.dram_tensor("xt", (TT, P, KS, P), dt8, kind="ExternalInput").ap()
    wt = nc.dram_tensor("wt", (P, KS, SHARD), dt8, kind="ExternalInput").ap()
    ya = nc.dram_tensor("ya", (TOKENS, SHARD), dt16, kind="ExternalInput").ap()
    y = nc.dram_tensor("y", (TOKENS, SHARD), f32, kind="ExternalOutput").ap()

    coff = [0]
    for ch in CHUNKS:
        coff.append(coff[-1] + ch)

    with tile.TileContext(nc) as tc, ExitStack() as ctx:
        wpool = ctx.enter_context(tc.tile_pool(name="w", bufs=1))
        xpool = ctx.enter_context(tc.tile_pool(name="x", bufs=4))
        opool = ctx.enter_context(tc.tile_pool(name="o", bufs=4))
        pspool = ctx.enter_context(tc.tile_pool(name="ps", bufs=2, space="PSUM"))

        w_sb = wpool.tile([P, KS, SHARD], dt8, name="w_sb")

        # PE prewarm: dependency-free dummy matmuls ramp the PE clock while
        # the first DMAs land.
        warm_in = wpool.tile([P, 2, P], dt8, name="warm_in")
        warm_mv = wpool.tile([P, 2, 256], dt8, name="warm_mv")
        nc.any.memzero(warm_in[:])
        nc.any.memzero(warm_mv[:])
        warm_ps = pspool.tile([P, 256], f32, name="warm_ps", tag="warm", bufs=1)
        for _ in range(25):
            nc.tensor.matmul(warm_ps[:], warm_in[:], warm_mv[:],
                             perf_mode=DRMODE, start=True, stop=True)

        # Early loads, interleaved in consumption order of the t0/t1 phase:
        # W slab s is consumed at DR step s//2, x ks-slice k at step k//2.
        # W dmas are split by column half to spread across more DMA queues.
        x0 = xpool.tile([P, KS, P], dt8, name="x_sb", tag="x_sb")
        x1 = xpool.tile([P, KS, P], dt8, name="x_sb", tag="x_sb")
        ya0 = xpool.tile([P, SHARD], dt16, name="ya_sb", tag="ya_sb")
        ya1 = xpool.tile([P, SHARD], dt16, name="ya_sb", tag="ya_sb")
        half = SHARD // 2

        sched = [(0, 2), (2, 4), (4, 8), (8, 12), (12, 16), (16, 20),
                 (20, 24), (24, 28), (28, KS)]
        for (a, b) in sched:
            nc.sync.dma_start(w_sb[:, a:b, :half], wt[:, a:b, :half])
            nc.sync.dma_start(w_sb[:, a:b, half:], wt[:, a:b, half:])
            nc.sync.dma_start(x0[:, a:b, :], xt[0, :, a:b, :])
            nc.sync.dma_start(x1[:, a:b, :], xt[1, :, a:b, :])
        nc.sync.dma_start(ya0[:], ya[0:P, :])
        nc.sync.dma_start(ya1[:], ya[P:2 * P, :])

        def eject_add(c, ps, ya_sb, o_sb):
            nc.vector.tensor_add(
                o_sb[:, coff[c]:coff[c + 1]], ps[:], ya_sb[:, coff[c]:coff[c + 1]]
            )

        # t = 0 and 1 interleaved over ks so combined compute covers the
        # W-load tail.
        pss01 = [
            [
                pspool.tile([P, CHUNKS[c]], f32, name=f"ps{c}", tag=f"ps{c}")
                for c in range(len(CHUNKS))
            ]
            for _ in range(2)
        ]
        for s in range(DR):
            for tt in range(2):
                x_sb = x0 if tt == 0 else x1
                for c in range(len(CHUNKS)):
                    nc.tensor.matmul(
                        pss01[tt][c][:],
                        x_sb[:, 2 * s:2 * s + 2, :],
                        w_sb[:, 2 * s:2 * s + 2, coff[c]:coff[c + 1]],
                        perf_mode=DRMODE,
                        start=(s == 0),
                        stop=(s == DR - 1),
                    )
        for tt in range(2):
            o_sb = opool.tile([P, SHARD], f32, name="o_sb", tag="o_sb")
            for c in range(len(CHUNKS)):
                eject_add(c, pss01[tt][c], ya0 if tt == 0 else ya1, o_sb)
            nc.sync.dma_start(y[tt * P:(tt + 1) * P, :], o_sb[:])

        def load_tile(t):
            x_sb = xpool.tile([P, KS, P], dt8, name="x_sb", tag="x_sb")
            ya_sb = xpool.tile([P, SHARD], dt16, name="ya_sb", tag="ya_sb")
            nc.sync.dma_start(x_sb[:], xt[t, :, :, :])
            nc.sync.dma_start(ya_sb[:], ya[t * P:(t + 1) * P, :])
            return x_sb, ya_sb

        nxt = load_tile(2)
        for t in range(2, TT):
            x_sb, ya_sb = nxt
            nxt = load_tile(t + 1) if t + 1 < TT else None

            pss = [
                pspool.tile([P, CHUNKS[c]], f32, name=f"ps{c}", tag=f"ps{c}")
                for c in range(len(CHUNKS))
            ]
            o_sb = opool.tile([P, SHARD], f32, name="o_sb", tag="o_sb")
            if t < TT - 1:
                for s in range(DR):
                    for c in range(len(CHUNKS)):
                        nc.tensor.matmul(
                            pss[c][:],
                            x_sb[:, 2 * s:2 * s + 2, :],
                            w_sb[:, 2 * s:2 * s + 2, coff[c]:coff[c + 1]],
                            perf_mode=DRMODE,
                            start=(s == 0),
                            stop=(s == DR - 1),
                        )
                    if s == 8 and nxt is not None:
                        # tiny matmul touching the next x tile: absorbs its
                        # DMA-done semaphore wait mid-stream so the next
                        # tile's first real matmul issues without a stall.
                        nc.tensor.matmul(
                            warm_ps[:, 0:1],
                            nxt[0][:, 0:2, :],
                            w_sb[:, 0:2, 0:1],
                            perf_mode=DRMODE,
                            start=True,
                            stop=True,
                        )
                for c in range(len(CHUNKS)):
                    eject_add(c, pss[c], ya_sb, o_sb)
                nc.sync.dma_start(y[t * P:(t + 1) * P, :], o_sb[:])
            else:
                # last tile chunk-major with per-chunk output DMA: each
                # chunk's eject and writeback overlap the next chunk's
                # matmuls, shortening the kernel tail.
                for c in range(len(CHUNKS)):
                    for s in range(DR):
                        nc.tensor.matmul(
                            pss[c][:],
                            x_sb[:, 2 * s:2 * s + 2, :],
                            w_sb[:, 2 * s:2 * s + 2, coff[c]:coff[c + 1]],
                            perf_mode=DRMODE,
                            start=(s == 0),
                            stop=(s == DR - 1),
                        )
                    eject_add(c, pss[c], ya_sb, o_sb)
                    nc.sync.dma_start(
                        y[t * P:(t + 1) * P, coff[c]:coff[c + 1]],
                        o_sb[:, coff[c]:coff[c + 1]],
                    )

    nc.compile()
    return nc


def _host_prep(x, w_packed, w_scale, w_bias, b):
    import ml_dtypes

    fp8 = ml_dtypes.float8_e4m3

    shifts = np.array([12, 8, 4, 0], dtype=np.int32)
    nib = ((w_packed[..., None] >> shifts) & 15).astype(np.float32)
    n_rows, n_groups, n_ids = w_packed.shape
    n = nib.reshape(n_rows, n_groups, n_ids * 4)         # (out, 64, 64)
    Wq = ((n - 7.5) * w_scale).reshape(n_rows, IN_F)     # residual (out, in)
    Tp = (7.5 * w_scale + w_bias)[..., 0]                # (out, 64)

    # exact group-affine part, computed in fp32 on host
    xg = x.reshape(TOKENS, NG, IN_F // NG).sum(axis=2)   # (tokens, 64)
    yaug = xg @ Tp.T + b[None, :]                        # (tokens, out)

    mx = np.abs(Wq).max(axis=1)
    mx = np.maximum(mx, 1e-30)
    A = np.exp2(np.floor(np.log2(128.0 / mx))).astype(np.float32)   # (out,)
    # keep the fp16-shipped yaug*A comfortably inside fp16 range
    ymax = np.abs(yaug).max(axis=0)
    bad = (ymax * A) > 50000.0
    while bad.any():
        A[bad] *= 0.5
        bad = (ymax * A) > 50000.0

    W8 = (Wq * A[:, None]).T.astype(fp8)                 # (in, out)
    # (P, KS, out): partition-major tile-blocked for multi-KB DMA runs
    W8b = np.ascontiguousarray(W8.reshape(KS, P, OUT_F).transpose(1, 0, 2))
    x8 = x.astype(fp8)                                   # (tokens, in)
    # (TT, P=k-in-slice, KS, P=token): 4KB contiguous per partition per tile
    x8b = np.ascontiguousarray(
        x8.reshape(TT, P, KS, P).transpose(0, 3, 2, 1)
    )
    yaugA = (yaug * A[None, :]).astype(np.float16)       # (tokens, out)

    in_maps = []
    for i in range(N_CORES):
        sl = slice(i * SHARD, (i + 1) * SHARD)
        in_maps.append(
            {
                "xt": x8b,
                "wt": np.ascontiguousarray(W8b[:, :, sl]),
                "ya": np.ascontiguousarray(yaugA[:, sl]),
            }
        )
    return in_maps, A


def _run(x, w_packed, w_scale, w_bias, b, trace=False):
    from concourse.bass_utils import run_bass_kernel_spmd

    if "nc" not in _NC_CACHE:
        _NC_CACHE["nc"] = _build_nc()
    nc = _NC_CACHE["nc"]
    in_maps, A = _host_prep(x, w_packed, w_scale, w_bias, b)
    res = run_bass_kernel_spmd(nc, in_maps, list(range(N_CORES)), trace=trace)
    y = np.concatenate([res.results[i]["y"] for i in range(N_CORES)], axis=1)
    y *= (1.0 / A)[None, :]
    return np.ascontiguousarray(y.astype(np.float32)), res


def kernel(x, w_packed, w_scale, w_bias, b):
    x = np.asarray(x)
    w_packed = np.asarray(w_packed)
    w_scale = np.asarray(w_scale)
    w_bias = np.asarray(w_bias)
    b = np.asarray(b)
    y, _ = _run(x, w_packed, w_scale, w_bias, b, trace=False)
    return y


# revision 16
# speedup vs baseline: 1.0118x; 1.0118x over previous
"""GroupQuantLinear: y = x @ dequant(w).T + b on 8 NeuronCores.

Strategy (column-parallel, fp8 DoubleRow PE path):
  The 4-bit weight W = n*s + t (nibble n in 0..15, per-group scale s, bias t)
  is split as W = (n-7.5)*s + (7.5*s + t).
    - Residual Wq = (n-7.5)*s carries only ~14% of W's variance. It is cast
      to fp8 e4m3 with a per-output-column power-of-2 scale A[o] (exactly
      invertible) and multiplied against fp8(x) using DoubleRow matmuls:
      2 K-slices (K=256) per PE instruction at 1 col/cycle -> 2x fp16 rate.
      Combined fp8 quantization error ~3.8% * sqrt(0.14) = ~1.4% rel.
    - The group-affine part y_aug = xg @ (7.5s+t).T + b (xg = per-group sums
      of x) is 1.5% of the FLOPs and exact; computed on host in fp32,
      shipped pre-scaled by A as fp16, and added to PSUM by the DVE during
      eject. Device returns A*y in fp32; host multiplies by 1/A.
  All DRAM tensors are laid out host-side so DMA descriptors are multi-KB
  contiguous runs (the DMA engines are descriptor-rate-bound ~40ns each).
  PE cost per (token-tile, chunk): 16 DoubleRow matmuls = 16N cycles vs
  32N for fp16 - the pure-fp8 PE floor (~587us/core).
  Shards W/outputs along out_features across 8 cores (1376 each).
"""

import os
import sys
from contextlib import ExitStack

import numpy as np

sys.path.insert(0, "/opt/trn_rl_repo")

TOKENS = 8192
IN_F = 4096
OUT_F = 11008
N_CORES = 8
SHARD = OUT_F // N_CORES          # 1376
CHUNKS = (512, 512, 352)          # out-cols per PSUM bank, sum = SHARD
P = 128
KS = IN_F // P                    # 32 k-slices
DR = KS // 2                      # 16 DoubleRow steps
TT = TOKENS // P                  # 64 token tiles
NG = 64                           # quant groups

_NC_CACHE = {}


def _build_nc():
    import concourse.bacc as bacc
    import concourse.mybir as mybir
    import concourse.tile as tile

    dt8 = mybir.dt.float8e4
    dt16 = mybir.dt.float16
    f32 = mybir.dt.float32
    DRMODE = mybir.MatmulPerfMode.DoubleRow

    nc = bacc.Bacc(
        "TRN2",
        target_bir_lowering=False,
        debug=False,
        enable_asserts=False,
        num_devices=N_CORES,
    )
    # tile-blocked layouts: contiguous multi-KB runs per partition
    xt = nc.dram_tensor("xt", (TT, P, KS, P), dt8, kind="ExternalInput").ap()
    wt = nc.dram_tensor("wt", (P, KS, SHARD), dt8, kind="ExternalInput").ap()
    ya = nc.dram_tensor("ya", (TOKENS, SHARD), dt16, kind="ExternalInput").ap()
    y = nc.dram_tensor("y", (TOKENS, SHARD), f32, kind="ExternalOutput").ap()

    coff = [0]
    for ch in CHUNKS:
        coff.append(coff[-1] + ch)

    with tile.TileContext(nc) as tc, ExitStack() as ctx:
        wpool = ctx.enter_context(tc.tile_pool(name="w", bufs=1))
        xpool = ctx.enter_context(tc.tile_pool(name="x", bufs=4))
        opool = ctx.enter_context(tc.tile_pool(name="o", bufs=4))
        pspool = ctx.enter_context(tc.tile_pool(name="ps", bufs=2, space="PSUM"))

        w_sb = wpool.tile([P, KS, SHARD], dt8, name="w_sb")

        # PE prewarm: dependency-free dummy matmuls ramp the PE clock while
        # the first DMAs land.
        warm_in = wpool.tile([P, 2, P], dt8, name="warm_in")
        warm_mv = wpool.tile([P, 2, 256], dt8, name="warm_mv")
        nc.any.memzero(warm_in[:])
        nc.any.memzero(warm_mv[:])
        warm_ps = pspool.tile([P, 256], f32, name="warm_ps", tag="warm", bufs=1)
        for _ in range(30):
            nc.tensor.matmul(warm_ps[:], warm_in[:], warm_mv[:],
                             perf_mode=DRMODE, start=True, stop=True)

        # Early loads, interleaved in consumption order of the t0/t1 phase:
        # W slab s is consumed at DR step s//2, x ks-slice k at step k//2.
        # W dmas are split by column half to spread across more DMA queues.
        x0 = xpool.tile([P, KS, P], dt8, name="x_sb", tag="x_sb")
        x1 = xpool.tile([P, KS, P], dt8, name="x_sb", tag="x_sb")
        ya0 = xpool.tile([P, SHARD], dt16, name="ya_sb", tag="ya_sb")
        ya1 = xpool.tile([P, SHARD], dt16, name="ya_sb", tag="ya_sb")
        sched = [(0, 2), (2, 4), (4, 8), (8, 16), (16, 24), (24, KS)]
        for (a, b) in sched:
            nc.sync.dma_start(w_sb[:, a:b, :], wt[:, a:b, :])
            nc.sync.dma_start(x0[:, a:b, :], xt[0, :, a:b, :])
            nc.sync.dma_start(x1[:, a:b, :], xt[1, :, a:b, :])
        nc.sync.dma_start(ya0[:], ya[0:P, :])
        nc.sync.dma_start(ya1[:], ya[P:2 * P, :])

        def eject_add(c, ps, ya_sb, o_sb):
            nc.vector.tensor_add(
                o_sb[:, coff[c]:coff[c + 1]], ps[:], ya_sb[:, coff[c]:coff[c + 1]]
            )

        # t = 0 and 1 interleaved over ks so combined compute covers the
        # W-load tail.
        pss01 = [
            [
                pspool.tile([P, CHUNKS[c]], f32, name=f"ps{c}", tag=f"ps{c}")
                for c in range(len(CHUNKS))
            ]
            for _ in range(2)
        ]
        for s in range(DR):
            for tt in range(2):
                x_sb = x0 if tt == 0 else x1
                for c in range(len(CHUNKS)):
                    nc.tensor.matmul(
                        pss01[tt][c][:],
                        x_sb[:, 2 * s:2 * s + 2, :],
                        w_sb[:, 2 * s:2 * s + 2, coff[c]:coff[c + 1]],
                        perf_mode=DRMODE,
                        start=(s == 0),
                        stop=(s == DR - 1),
                    )
        for tt in range(2):
            o_sb = opool.tile([P, SHARD], f32, name="o_sb", tag="o_sb")
            for c in range(len(CHUNKS)):
                eject_add(c, pss01[tt][c], ya0 if tt == 0 else ya1, o_sb)
            nc.sync.dma_start(y[tt * P:(tt + 1) * P, :], o_sb[:])

        def load_tile(t):
            x_sb = xpool.tile([P, KS, P], dt8, name="x_sb", tag="x_sb")
            ya_sb = xpool.tile([P, SHARD], dt16, name="ya_sb", tag="ya_sb")
            nc.sync.dma_start(x_sb[:], xt[t, :, :, :])
            nc.sync.dma_start(ya_sb[:], ya[t * P:(t + 1) * P, :])
            return x_sb, ya_sb

        nxt = load_tile(2)
        for t in range(2, TT):
            x_sb, ya_sb = nxt
            nxt = load_tile(t + 1) if t + 1 < TT else None

            pss = [
                pspool.tile([P, CHUNKS[c]], f32, name=f"ps{c}", tag=f"ps{c}")
                for c in range(len(CHUNKS))
            ]
            o_sb = opool.tile([P, SHARD], f32, name="o_sb", tag="o_sb")
            if t < TT - 1:
                for s in range(DR):
                    for c in range(len(CHUNKS)):
                        nc.tensor.matmul(
                            pss[c][:],
                            x_sb[:, 2 * s:2 * s + 2, :],
                            w_sb[:, 2 * s:2 * s + 2, coff[c]:coff[c + 1]],
                            perf_mode=DRMODE,
                            start=(s == 0),
                            stop=(s == DR - 1),
                        )
                for c in range(len(CHUNKS)):
                    eject_add(c, pss[c], ya_sb, o_sb)
                nc.sync.dma_start(y[t * P:(t + 1) * P, :], o_sb[:])
            else:
                # last tile chunk-major: each chunk ejects while the next
                # one is still on the PE, shortening the kernel tail.
                for c in range(len(CHUNKS)):
                    for s in range(DR):
                        nc.tensor.matmul(
                            pss[c][:],
                            x_sb[:, 2 * s:2 * s + 2, :],
                            w_sb[:, 2 * s:2 * s + 2, coff[c]:coff[c + 1]],
                            perf_mode=DRMODE,
                            start=(s == 0),
                            stop=(s == DR - 1),
                        )
                    eject_add(c, pss[c], ya_sb, o_sb)
                nc.sync.dma_start(y[t * P:(t + 1) * P, :], o_sb[:])

    nc.compile()
    return nc


def _host_prep(x, w_packed, w_scale, w_bias, b):
    import ml_dtypes

    fp8 = ml_dtypes.float8_e4m3

    shifts = np.array([12, 8, 4, 0], dtype=np.int32)
    nib = ((w_packed[..., None] >> shifts) & 15).astype(np.float32)
    n_rows, n_groups, n_ids = w_packed.shape
    n = nib.reshape(n_rows, n_groups, n_ids * 4)         # (out, 64, 64)
    Wq = ((n - 7.5) * w_scale).reshape(n_rows, IN_F)     # residual (out, in)
    Tp = (7.5 * w_scale + w_bias)[..., 0]                # (out, 64)

    # exact group-affine part, computed in fp32 on host
    xg = x.reshape(TOKENS, NG, IN_F // NG).sum(axis=2)   # (tokens, 64)
    yaug = xg @ Tp.T + b[None, :]                        # (tokens, out)

    mx = np.abs(Wq).max(axis=1)
    mx = np.maximum(mx, 1e-30)
    A = np.exp2(np.floor(np.log2(128.0 / mx))).astype(np.float32)   # (out,)
    # keep the fp16-shipped yaug*A comfortably inside fp16 range
    ymax = np.abs(yaug).max(axis=0)
    bad = (ymax * A) > 50000.0
    while bad.any():
        A[bad] *= 0.5
        bad = (ymax * A) > 50000.0

    W8 = (Wq * A[:, None]).T.astype(fp8)                 # (in, out)
    # (P, KS, out): partition-major tile-blocked for multi-KB DMA runs
    W8b = np.ascontiguousarray(W8.reshape(KS, P, OUT_F).transpose(1, 0, 2))
    x8 = x.astype(fp8)                                   # (tokens, in)
    # (TT, P=k-in-slice, KS, P=token): 4KB contiguous per partition per tile
    x8b = np.ascontiguousarray(
        x8.reshape(TT, P, KS, P).transpose(0, 3, 2, 1)
    )
    yaugA = (yaug * A[None, :]).astype(np.float16)       # (tokens, out)

    in_maps = []
    for i in range(N_CORES):
        sl = slice(i * SHARD, (i + 1) * SHARD)
        in_maps.append(
            {
                "xt": x8b,
                "wt": np.ascontiguousarray(W8b[:, :, sl]),
                "ya": np.ascontiguousarray(yaugA[:, sl]),
            }
        )
    return in_maps, A


def _run(x, w_packed, w_scale, w_bias, b, trace=False):
    from concourse.bass_utils import run_bass_kernel_spmd

    if "nc" not in _NC_CACHE:
        _NC_CACHE["nc"] = _build_nc()
    nc = _NC_CACHE["nc"]
    in_maps, A = _host_prep(x, w_packed, w_scale, w_bias, b)
    res = run_bass_kernel_spmd(nc, in_maps, list(range(N_CORES)), trace=trace)
    y = np.concatenate([res.results[i]["y"] for i in range(N_CORES)], axis=1)
    y *= (1.0 / A)[None, :]
    return np.ascontiguousarray(y.astype(np.float32)), res


def kernel(x, w_packed, w_scale, w_bias, b):
    x = np.asarray(x)
    w_packed = np.asarray(w_packed)
    w_scale = np.asarray(w_scale)
    w_bias = np.asarray(w_bias)
    b = np.asarray(b)
    y, _ = _run(x, w_packed, w_scale, w_bias, b, trace=False)
    return y
